# revision 7
# baseline (speedup 1.0000x reference)
"""Trainium2 Bass kernel for windowed (sparse) Bahdanau attention.

Reference computation (per batch b):
    window  = values[start_b : start_b+128, b, :]   (clipped to [0,T), zero-padded)
    qb1     = query[b] @ W2 + W1_b + W2_b
    score_l = V . tanh(window_l @ W1 + qb1)         (V_b cancels in softmax)
    w       = softmax(score)                        -> attention_weights output
    ctx     = sum_l w_l * window_l                  -> context_vector output

Strategy: data-parallel over batch across 8 NeuronCores (16 batches/core).
On each core the windows are fetched from HBM with two SWDGE row gathers of
the bf16-cast values shard: one transposed gather that lands the windows with
the D axis on partitions (ready to be the moving operand of the W1 matmul,
no on-chip transposes), and one natural gather ([l, d] rows) for the final
context matmul. Invalid (clipped / out-of-range) window rows are never
masked in the gather; instead their scores are patched to the exact
"zero window" score V.tanh(qb1) and their softmax weights are zeroed before
the context matmul, which reproduces the reference's zero-padding semantics
exactly.
"""

import numpy as np
import ml_dtypes

from contextlib import ExitStack

import concourse.bass as bass
import concourse.bacc as bacc
import concourse.tile as tile
from concourse import mybir
from concourse.bass_utils import run_bass_kernel_spmd

B, T, D, U, L = 128, 2048, 512, 512, 128
NCORES = 8
BL = B // NCORES          # 16 local batches per core
DC = D // 128             # 4 contraction chunks
UC = U // 128             # 4 units chunks
NG = BL // 4              # 4 batch groups of 4 (N=512 matmul slices)
HALF = L // 2

bf16 = mybir.dt.bfloat16
f32 = mybir.dt.float32
i16 = mybir.dt.int16
npbf = ml_dtypes.bfloat16
AF = mybir.ActivationFunctionType
ALU = mybir.AluOpType
AX = mybir.AxisListType


def _build_nc():
    nc = bacc.Bacc("TRN2", target_bir_lowering=False, debug=False,
                   num_devices=NCORES)

    vrows = nc.dram_tensor("vrows", [T * BL, D], bf16, kind="ExternalInput").ap()
    idxt = nc.dram_tensor("idxt", [128, 128], i16, kind="ExternalInput").ap()
    w1p = nc.dram_tensor("w1p", [128, DC * U], bf16, kind="ExternalInput").ap()
    w2p = nc.dram_tensor("w2p", [128, DC * U], bf16, kind="ExternalInput").ap()
    qtp = nc.dram_tensor("qtp", [128, DC * BL], bf16, kind="ExternalInput").ap()
    vbp = nc.dram_tensor("vbp", [128, UC], bf16, kind="ExternalInput").ap()
    bsp = nc.dram_tensor("bsp", [1, U], bf16, kind="ExternalInput").ap()
    mkp = nc.dram_tensor("mkp", [BL, L], f32, kind="ExternalInput").ap()
    idp = nc.dram_tensor("idp", [BL, BL], bf16, kind="ExternalInput").ap()
    out_c = nc.dram_tensor("out_ctx", [BL, D], f32, kind="ExternalOutput").ap()
    out_w = nc.dram_tensor("out_w", [BL, L], f32, kind="ExternalOutput").ap()

    with tile.TileContext(nc) as tc, ExitStack() as ctx:
        const = ctx.enter_context(tc.tile_pool(name="const", bufs=1))

        w1 = const.tile([128, DC * U], bf16)
        nc.sync.dma_start(w1[:], w1p)
        w2 = const.tile([128, DC * U], bf16)
        nc.sync.dma_start(w2[:], w2p)
        qt = const.tile([128, DC * BL], bf16)
        nc.sync.dma_start(qt[:], qtp)
        vb = const.tile([128, UC], bf16)
        nc.sync.dma_start(vb[:], vbp)
        bs = const.tile([1, U], bf16)
        nc.sync.dma_start(bs[:], bsp)
        mk = const.tile([BL, L], f32)
        nc.sync.dma_start(mk[:], mkp)
        idt = const.tile([128, 128], i16)
        nc.sync.dma_start(idt[:], idxt)
        idn = const.tile([BL, BL], bf16)
        nc.sync.dma_start(idn[:], idp)
        ones = const.tile([1, BL], bf16)
        nc.vector.memset(ones[:], 1.0)

        # windowed gathers: 2048 rows (16 batches x 128 window positions).
        # Split to keep each call within the single-packet descriptor limit
        # (transposed gathers >512 idxs / natural >1024 idxs crash the
        # exec unit with single_packet=True, and single_packet=False fires
        # the completion semaphore before the later packets finish).
        gat = ctx.enter_context(tc.tile_pool(name="gather", bufs=1))
        gTs = []   # per batch-group n: [d%128, (d//128, 4*128 (b,l))]
        for n in range(NG):
            gt = gat.tile([128, DC * 512], bf16, tag=f"gT{n}")
            nc.gpsimd.dma_gather(
                gt[:].rearrange("p (j n) -> p j n", j=DC), vrows,
                idt[:, n * 32:(n + 1) * 32], 512, 512, D, transpose=True)
            gTs.append(gt[:].rearrange("p (j n) -> p j n", j=DC))
        gNs = []   # per half h: [l, (b in 8, d)]
        for h in range(2):
            gn = gat.tile([128, 8 * D], bf16, tag=f"gN{h}")
            nc.gpsimd.dma_gather(
                gn[:].rearrange("p (b d) -> p b d", b=8), vrows,
                idt[:, h * 64:(h + 1) * 64], 1024, 1024, D)
            gNs.append(gn[:].rearrange("p (b d) -> p b d", b=8))

        # ---- q projection: qb1T[u, b] = sum_d W2[d,u] q[b,d] + (W1_b+W2_b)[u]
        qb1 = const.tile([128, UC * BL], f32)
        with tc.tile_pool(name="psq", bufs=2, space="PSUM") as psq:
            for i in range(UC):
                pq = psq.tile([128, BL], f32, tag="q")
                for j in range(DC):
                    nc.tensor.matmul(pq[:],
                                     w2[:, j * U + i * 128:j * U + i * 128 + 128],
                                     qt[:, j * BL:(j + 1) * BL],
                                     start=(j == 0), stop=False)
                nc.tensor.matmul(pq[:], bs[0:1, i * 128:(i + 1) * 128],
                                 ones[0:1, :], start=False, stop=True)
                nc.any.tensor_copy(qb1[:, i * BL:(i + 1) * BL], pq[:])

        xp = ctx.enter_context(tc.tile_pool(name="xpsum", bufs=3, space="PSUM"))
        sp = ctx.enter_context(tc.tile_pool(name="spsum", bufs=2, space="PSUM"))
        cp = ctx.enter_context(tc.tile_pool(name="cpsum", bufs=2, space="PSUM"))
        xs = ctx.enter_context(tc.tile_pool(name="xsb", bufs=4))
        th = ctx.enter_context(tc.tile_pool(name="tanh", bufs=8))
        sm = ctx.enter_context(tc.tile_pool(name="small", bufs=2))

        # scores staging row: 16 batches x (128 scores + 1 invalid-score)
        s_row = const.tile([1, BL * (L + 1)], f32)
        s_row_v = s_row[:].rearrange("o (b e) -> o b e", b=BL)

        # ---- main matmul X.T[u, (b,l)] + qb1 broadcast + tanh + V-dot
        for n in range(NG):
            tanhs = []
            for i in range(UC):
                px = xp.tile([128, 512], f32, tag="x")
                for j in range(DC):
                    nc.tensor.matmul(px[:],
                                     w1[:, j * U + i * 128:j * U + i * 128 + 128],
                                     gTs[n][:, j, :],
                                     start=(j == 0), stop=(j == DC - 1))
                xb = xs.tile([128, 512], f32, tag="xb")
                qsl = qb1[:, i * BL + 4 * n:i * BL + 4 * n + 4]
                nc.vector.tensor_tensor(
                    xb[:].rearrange("p (b l) -> p b l", b=4),
                    px[:].rearrange("p (b l) -> p b l", b=4),
                    qsl.unsqueeze(-1).broadcast_to((128, 4, 128)),
                    op=ALU.add)
                tt = th.tile([128, 512], bf16, tag="t")
                nc.scalar.activation(tt[:], xb[:], AF.Tanh)
                tanhs.append(tt)
            ps = sp.tile([1, 512], f32, tag="s")
            for i in range(UC):
                nc.tensor.matmul(ps[:], vb[:, i:i + 1], tanhs[i][:],
                                 start=(i == 0), stop=(i == UC - 1))
            nc.any.tensor_copy(
                s_row_v[0:1, 4 * n:4 * n + 4, 0:L],
                ps[:].rearrange("o (b l) -> o b l", b=4))

        # ---- invalid-row score: V . tanh(qb1)
        tq = sm.tile([128, UC * BL], bf16, tag="tq")
        nc.scalar.activation(tq[:], qb1[:], AF.Tanh)
        psv = sp.tile([1, BL], f32, tag="s")
        for i in range(UC):
            nc.tensor.matmul(psv[:], vb[:, i:i + 1], tq[:, i * BL:(i + 1) * BL],
                             start=(i == 0), stop=(i == UC - 1))
        nc.any.tensor_copy(s_row_v[0:1, :, L:L + 1],
                           psv[:].rearrange("o (b e) -> o b e", b=BL))

        # ---- relayout scores to [b, l] and run softmax
        s_bt = sm.tile([BL, L + 1], f32, tag="sbt")
        nc.sync.dma_start(s_bt[:], s_row[:])
        sb = s_bt[:, 0:L]
        sv = s_bt[:, L:L + 1]

        spat = sm.tile([BL, L], f32, tag="spat")
        # (s - sinv) * mask
        nc.vector.scalar_tensor_tensor(spat[:], sb, sv, mk[:],
                                       op0=ALU.subtract, op1=ALU.mult)
        # + sinv
        nc.vector.tensor_scalar(spat[:], spat[:], sv, None, op0=ALU.add)

        mx = sm.tile([BL, 1], f32, tag="mx")
        nc.vector.tensor_reduce(mx[:], spat[:], axis=AX.X, op=ALU.max)
        nmx = sm.tile([BL, 1], f32, tag="nmx")
        nc.vector.tensor_scalar(nmx[:], mx[:], -1.0, None, op0=ALU.mult)
        ew = sm.tile([BL, L], f32, tag="ew")
        den = sm.tile([BL, 1], f32, tag="den")
        nc.scalar.activation(ew[:], spat[:], AF.Exp, bias=nmx[:], scale=1.0,
                             accum_out=den[:])
        rden = sm.tile([BL, 1], f32, tag="rden")
        nc.vector.reciprocal(rden[:], den[:])
        w_t = sm.tile([BL, L], f32, tag="wt")
        nc.vector.tensor_scalar(w_t[:], ew[:], rden[:], None, op0=ALU.mult)
        nc.sync.dma_start(out_w, w_t[:])

        # ---- masked weights, transposed, drive the context matmul
        wm = sm.tile([BL, L], bf16, tag="wm")
        nc.vector.tensor_tensor(wm[:], w_t[:], mk[:], op=ALU.mult)
        pwt = cp.tile([128, BL], bf16, tag="cw", bufs=1)
        nc.tensor.transpose(pwt[:], wm[:], idn[:])
        wT = sm.tile([128, BL], bf16, tag="wT")
        nc.any.tensor_copy(wT[:], pwt[:])

        for b in range(BL):
            pc = cp.tile([1, D], f32, tag="c")
            nc.tensor.matmul(pc[:], wT[:, b:b + 1], gNs[b // 8][:, b % 8, :],
                             start=True, stop=True)
            cs = sm.tile([1, D], f32, tag="cs", bufs=4)
            nc.any.tensor_copy(cs[:], pc[:])
            nc.sync.dma_start(out_c[b:b + 1, :], cs[:])

    nc.compile()
    return nc


_NC_CACHE = {}


def _get_nc():
    if "nc" not in _NC_CACHE:
        _NC_CACHE["nc"] = _build_nc()
    return _NC_CACHE["nc"]


def _prep_shared(query, W1_w, W1_b, W2_w, W2_b, V_w, pos):
    """Small-tensor host prep shared across cores."""
    w1p = np.ascontiguousarray(
        W1_w.astype(npbf).reshape(DC, 128, U).transpose(1, 0, 2).reshape(128, DC * U))
    w2p = np.ascontiguousarray(
        W2_w.astype(npbf).reshape(DC, 128, U).transpose(1, 0, 2).reshape(128, DC * U))
    vbp = np.ascontiguousarray(V_w.reshape(UC, 128).astype(npbf).T)
    bsp = (W1_b + W2_b).astype(npbf).reshape(1, U)
    idp = np.eye(BL, dtype=npbf)

    start = np.clip(pos.astype(np.int64) - HALF, 0, None)
    end = np.minimum(pos.astype(np.int64) + HALF, T)
    nvalid = np.clip(end - start, 0, L)
    lar = np.arange(L)
    tidx = np.clip(start[:, None] + lar[None, :], 0, T - 1)      # [B, L]
    mask = (lar[None, :] < nvalid[:, None]).astype(np.float32)   # [B, L]
    return w1p, w2p, vbp, bsp, idp, tidx, mask


def kernel(**inputs):
    nc = _get_nc()

    query = np.asarray(inputs["query"], dtype=np.float32)
    values = np.asarray(inputs["values"], dtype=np.float32)
    pos = np.asarray(inputs["pos"])
    W1_w = np.asarray(inputs["W1_w"], dtype=np.float32)
    W1_b = np.asarray(inputs["W1_b"], dtype=np.float32)
    W2_w = np.asarray(inputs["W2_w"], dtype=np.float32)
    W2_b = np.asarray(inputs["W2_b"], dtype=np.float32)
    V_w = np.asarray(inputs["V_w"], dtype=np.float32)

    w1p, w2p, vbp, bsp, idp, tidx, mask = _prep_shared(
        query, W1_w, W1_b, W2_w, W2_b, V_w, pos)

    vbf = values.astype(npbf)                                    # [T, B, D]
    bloc = np.arange(BL)

    in_maps = []
    for c in range(NCORES):
        bsl = slice(c * BL, (c + 1) * BL)
        vr = np.ascontiguousarray(vbf[:, bsl, :]).reshape(T * BL, D)
        rows = (tidx[bsl, :] * BL + bloc[:, None]).astype(np.int16)  # [16,128]
        flat = rows.reshape(-1)                                  # i = b*128 + l
        table = np.ascontiguousarray(flat.reshape(128, 16).T)    # [16, 128]
        table128 = np.tile(table, (8, 1))                        # replicated
        q = query[bsl].astype(npbf)                              # [16, 512]
        qtp = np.ascontiguousarray(
            q.reshape(BL, DC, 128).transpose(2, 1, 0).reshape(128, DC * BL))
        in_maps.append({
            "vrows": vr,
            "idxt": table128,
            "w1p": w1p,
            "w2p": w2p,
            "qtp": qtp,
            "vbp": vbp,
            "bsp": bsp,
            "mkp": np.ascontiguousarray(mask[bsl]),
            "idp": idp,
        })

    res = run_bass_kernel_spmd(nc, in_maps, list(range(NCORES))).results
    ctxv = np.concatenate([r["out_ctx"] for r in res], axis=0)       # [B, D]
    attw = np.concatenate([r["out_w"] for r in res], axis=0)[..., None]
    return ctxv, attw


# revision 15
# speedup vs baseline: 1.2399x; 1.2399x over previous
"""Trainium2 Bass kernel for windowed (sparse) Bahdanau attention.

Reference computation (per batch b):
    window  = values[start_b : start_b+128, b, :]   (clipped to [0,T), zero-padded)
    qb1     = query[b] @ W2 + W1_b + W2_b
    score_l = V . tanh(window_l @ W1 + qb1)         (V_b cancels in softmax)
    w       = softmax(score)                        -> attention_weights output
    ctx     = sum_l w_l * window_l                  -> context_vector output

Strategy: data-parallel over batch across 8 NeuronCores (16 batches/core).
On each core the windows are fetched from HBM with SWDGE row gathers of the
bf16-cast values shard: transposed gathers land the windows with the D axis
on partitions (ready to be the moving operand of the W1 matmul, no on-chip
transposes), natural gathers provide [l, d] rows for the final context
matmul. Invalid (clipped / out-of-range) window rows are never masked in the
gather; instead their scores are patched to the exact "zero window" score
V.tanh(qb1) and their softmax weights are zeroed before the context matmul,
which reproduces the reference's zero-padding semantics exactly.

Score and context matmuls are M=1 rank-1 reductions; they are packed four to
a PSUM bank with tile_position col-tiling so their results can be drained
with a couple of wide [128, 512] copies and single DMAs instead of
single-partition ops.
"""

import os

import numpy as np
import ml_dtypes

from contextlib import ExitStack

import concourse.bass as bass
import concourse.bacc as bacc
import concourse.tile as tile
from concourse import mybir
from concourse.bass_utils import run_bass_kernel_spmd

B, T, D, U, L = 128, 2048, 512, 512, 128
NCORES = 8
BL = B // NCORES          # 16 local batches per core
DC = D // 128             # 4 contraction chunks
UC = U // 128             # 4 units chunks
NG = BL // 4              # 4 batch groups of 4 (N=512 matmul slices)
HALF = L // 2

# packed bf16 constants tensor layout (columns)
W1_OFF, W2_OFF = 0, 2048
QT_OFF, VB_OFF = 4096, 4160
IDN_OFF, BS_OFF, ONE_OFF = 4164, 4180, 4692
Z_OFF = 4708
WP_COLS = 4836

bf16 = mybir.dt.bfloat16
f32 = mybir.dt.float32
i16 = mybir.dt.int16
npbf = ml_dtypes.bfloat16
AF = mybir.ActivationFunctionType
ALU = mybir.AluOpType
AX = mybir.AxisListType


def _build_nc(reps=1):
    nc = bacc.Bacc("TRN2", target_bir_lowering=False, debug=False,
                   num_devices=NCORES)

    vrows = nc.dram_tensor("vrows", [T * BL, D], bf16, kind="ExternalInput").ap()
    idxt = nc.dram_tensor("idxt", [128, 128], i16, kind="ExternalInput").ap()
    wpk = nc.dram_tensor("wpk", [128, WP_COLS], bf16, kind="ExternalInput").ap()
    mkp = nc.dram_tensor("mkp", [BL, L], f32, kind="ExternalInput").ap()
    out_c = nc.dram_tensor("out_ctx", [BL, D], f32, kind="ExternalOutput").ap()
    out_w = nc.dram_tensor("out_w", [BL, L], f32, kind="ExternalOutput").ap()

    bench_loop = int(os.environ.get("BENCH_LOOP", "0"))

    with tile.TileContext(nc) as tc, ExitStack() as ctx:
        const = ctx.enter_context(tc.tile_pool(name="const", bufs=1))
        wp = const.tile([128, WP_COLS], bf16)
        nc.sync.dma_start(wp[:], wpk)
        mk = const.tile([BL, L], f32)
        nc.sync.dma_start(mk[:], mkp)
        idt = const.tile([128, 128], i16)
        nc.sync.dma_start(idt[:], idxt)

        w1 = wp[:, W1_OFF:W1_OFF + DC * U]
        w2 = wp[:, W2_OFF:W2_OFF + DC * U]
        qt = wp[:, QT_OFF:QT_OFF + DC * BL]
        vb = wp[:, VB_OFF:VB_OFF + UC]
        idn = wp[0:BL, IDN_OFF:IDN_OFF + BL]
        ones = wp[0:1, ONE_OFF:ONE_OFF + BL]
        zrow = wp[0:1, Z_OFF:Z_OFF + 128]

        pools = dict(
            gat=ctx.enter_context(tc.tile_pool(name="gather", bufs=1)),
            qb1p=ctx.enter_context(tc.tile_pool(name="qb1", bufs=1)),
            psq=ctx.enter_context(tc.tile_pool(name="psq", bufs=1, space="PSUM")),
            xp=ctx.enter_context(tc.tile_pool(name="xpsum", bufs=4, space="PSUM")),
            sp=ctx.enter_context(tc.tile_pool(name="spsum", bufs=1, space="PSUM")),
            cp=ctx.enter_context(tc.tile_pool(name="cpsum", bufs=1, space="PSUM")),
            xs=ctx.enter_context(tc.tile_pool(name="xsb", bufs=4)),
            th=ctx.enter_context(tc.tile_pool(name="tanh", bufs=8)),
            sm=ctx.enter_context(tc.tile_pool(name="small", bufs=2)),
        )
        args = (nc, tc, pools, vrows, idt, w1, w2, qt, vb, idn, ones, zrow,
                wp, mk, out_c, out_w)
        if bench_loop:
            with tc.For_i(0, bench_loop, 1):
                _body(*args)
        else:
            for _ in range(reps):
                _body(*args)

    nc.compile()
    return nc


def _body(nc, tc, pools, vrows, idt, w1, w2, qt, vb, idn, ones, zrow,
          wp, mk, out_c, out_w):
    gat, qb1p, psq = pools["gat"], pools["qb1p"], pools["psq"]
    xp, sp, cp = pools["xp"], pools["sp"], pools["cp"]
    xs, th, sm = pools["xs"], pools["th"], pools["sm"]
    # windowed gathers: 2048 rows (16 batches x 128 window positions).
    # Split to keep each call within the single-packet descriptor limit
    # (transposed gathers >512 idxs / natural >1024 idxs crash the exec
    # unit with single_packet=True, and single_packet=False fires the
    # completion semaphore before the later packets finish).
    gTs = []   # per batch-group n: [d%128, (d//128, 4*128 (b,l))]
    for n in range(NG):
        gt = gat.tile([128, DC * 512], bf16, tag=f"gT{n}")
        nc.gpsimd.dma_gather(
            gt[:].rearrange("p (j n) -> p j n", j=DC), vrows,
            idt[:, n * 32:(n + 1) * 32], 512, 512, D, transpose=True)
        gTs.append(gt[:].rearrange("p (j n) -> p j n", j=DC))
    gNs = []   # per half h: [l, (b in 8, d)]
    for h in range(2):
        gn = gat.tile([128, 8 * D], bf16, tag=f"gN{h}")
        nc.gpsimd.dma_gather(
            gn[:].rearrange("p (b d) -> p b d", b=8), vrows,
            idt[:, h * 64:(h + 1) * 64], 1024, 1024, D)
        gNs.append(gn[:].rearrange("p (b d) -> p b d", b=8))

    # ---- q projection: qb1T[u, b] = sum_d W2[d,u] q[b,d] + (W1_b+W2_b)[u]
    qb1 = qb1p.tile([128, UC * BL], f32, tag="qb1")
    for i in range(UC):
        pq = psq.tile([128, BL], f32, tag="q")
        for j in range(DC):
            nc.tensor.matmul(pq[:],
                             w2[:, j * U + i * 128:j * U + i * 128 + 128],
                             qt[:, j * BL:(j + 1) * BL],
                             start=(j == 0), stop=False)
        nc.tensor.matmul(pq[:], wp[0:1, BS_OFF + i * 128:BS_OFF + (i + 1) * 128],
                         ones, start=False, stop=True)
        nc.any.tensor_copy(qb1[:, i * BL:(i + 1) * BL], pq[:])

    # ---- main matmul X.T[u, (b,l)] + qb1 broadcast + tanh; col-tiled V-dot
    ps4 = sp.tile([128, 512], f32, tag="s")   # scores: group n at partition 32n
    # zero-fill the bank so the col-tiled M=1 V-dots can accumulate into it
    # and the wide drain copy reads fully-initialized PSUM
    nc.tensor.matmul(ps4[:], zrow, wp[0:1, 0:512], start=True, stop=False,
                     skip_group_check=True)
    for n in range(NG):
        tanhs = []
        for i in range(UC):
            px = xp.tile([128, 512], f32, tag="x")
            for j in range(DC):
                nc.tensor.matmul(px[:],
                                 w1[:, j * U + i * 128:j * U + i * 128 + 128],
                                 gTs[n][:, j, :],
                                 start=(j == 0), stop=(j == DC - 1))
            xb = xs.tile([128, 512], f32, tag="xb")
            qsl = qb1[:, i * BL + 4 * n:i * BL + 4 * n + 4]
            nc.vector.tensor_tensor(
                xb[:].rearrange("p (b l) -> p b l", b=4),
                px[:].rearrange("p (b l) -> p b l", b=4),
                qsl.unsqueeze(-1).broadcast_to((128, 4, 128)),
                op=ALU.add)
            tt = th.tile([128, 512], bf16, tag="t")
            nc.scalar.activation(tt[:], xb[:], AF.Tanh)
            tanhs.append(tt)
        for i in range(UC):
            nc.tensor.matmul(ps4[32 * n:32 * n + 1, :], vb[:, i:i + 1],
                             tanhs[i][:], start=False,
                             stop=(n == NG - 1 and i == UC - 1),
                             tile_position=(0, 32 * n),
                             skip_group_check=True)

    # ---- invalid-row score: V . tanh(qb1)
    tq = sm.tile([128, UC * BL], bf16, tag="tq")
    nc.scalar.activation(tq[:], qb1[:], AF.Tanh)
    psv = sp.tile([1, BL], f32, tag="sv", bufs=1)
    for i in range(UC):
        nc.tensor.matmul(psv[:], vb[:, i:i + 1], tq[:, i * BL:(i + 1) * BL],
                         start=(i == 0), stop=(i == UC - 1))

    # ---- relayout scores to [b, l] (+ sinv in col 128) and run softmax
    s4 = sm.tile([128, 512], f32, tag="s4")
    nc.any.tensor_copy(s4[:], ps4[:])
    sveq = sm.tile([1, BL], f32, tag="sveq")
    nc.any.tensor_copy(sveq[:], psv[:])

    s_bt = sm.tile([BL, L + 1], f32, tag="sbt")
    nc.sync.dma_start(
        s_bt[:, 0:L],
        s4[:].rearrange("(q s) (b l) -> q s b l", s=32, b=4)[:, 0, :, :])
    nc.sync.dma_start(s_bt[:, L:L + 1],
                      sveq[:].rearrange("o (b e) -> o b e", b=BL))
    sb = s_bt[:, 0:L]
    sv = s_bt[:, L:L + 1]

    spat = sm.tile([BL, L], f32, tag="spat")
    # (s - sinv) * mask + sinv  ==  mask ? s : sinv
    nc.vector.scalar_tensor_tensor(spat[:], sb, sv, mk[:],
                                   op0=ALU.subtract, op1=ALU.mult)
    nc.vector.tensor_scalar(spat[:], spat[:], sv, None, op0=ALU.add)

    mx = sm.tile([BL, 1], f32, tag="mx")
    nc.vector.tensor_reduce(mx[:], spat[:], axis=AX.X, op=ALU.max)
    nmx = sm.tile([BL, 1], f32, tag="nmx")
    nc.vector.tensor_scalar(nmx[:], mx[:], -1.0, None, op0=ALU.mult)
    ew = sm.tile([BL, L], f32, tag="ew")
    den = sm.tile([BL, 1], f32, tag="den")
    nc.scalar.activation(ew[:], spat[:], AF.Exp, bias=nmx[:], scale=1.0,
                         accum_out=den[:])
    rden = sm.tile([BL, 1], f32, tag="rden")
    nc.vector.reciprocal(rden[:], den[:])
    w_t = sm.tile([BL, L], f32, tag="wt")
    nc.vector.tensor_scalar(w_t[:], ew[:], rden[:], None, op0=ALU.mult)
    nc.sync.dma_start(out_w, w_t[:])

    # ---- masked weights, transposed, drive the col-tiled context matmuls
    wm = sm.tile([BL, L], bf16, tag="wm")
    nc.vector.tensor_tensor(wm[:], w_t[:], mk[:], op=ALU.mult)
    pwt = cp.tile([128, BL], bf16, tag="c", bufs=1)
    nc.tensor.transpose(pwt[:], wm[:], idn)
    wT = sm.tile([128, BL], bf16, tag="wT")
    nc.any.tensor_copy(wT[:], pwt[:])

    # batch b -> quad tile g = b % 4, partition 32*(b // 4)
    cstage = sm.tile([128, NG * 512], f32, tag="cst", bufs=1)
    for g in range(4):
        pc = cp.tile([128, 512], f32, tag="c", bufs=1)
        nc.tensor.matmul(pc[:], zrow, wp[0:1, 0:512], start=True, stop=False,
                         skip_group_check=True)
        for c in range(4):
            b = 4 * c + g
            nc.tensor.matmul(pc[32 * c:32 * c + 1, :], wT[:, b:b + 1],
                             gNs[b // 8][:, b % 8, :],
                             start=False, stop=(c == 3),
                             tile_position=(0, 32 * c),
                             skip_group_check=True)
        nc.any.tensor_copy(cstage[:, g * 512:(g + 1) * 512], pc[:])
    nc.sync.dma_start(
        out_c,
        cstage[:].rearrange("(c s) (g d) -> c s g d", s=32, g=4)[:, 0, :, :])


_NC_CACHE = {}


def _get_nc():
    if "nc" not in _NC_CACHE:
        _NC_CACHE["nc"] = _build_nc(int(os.environ.get("KERNEL_REPS", "1")))
    return _NC_CACHE["nc"]


def _prep_shared(query, W1_w, W1_b, W2_w, W2_b, V_w, pos):
    """Small-tensor host prep shared across cores. Returns the packed
    constants tensor minus the per-core qt columns, plus gather metadata."""
    wpk = np.zeros((128, WP_COLS), dtype=npbf)
    wpk[:, W1_OFF:W1_OFF + DC * U] = (
        W1_w.astype(npbf).reshape(DC, 128, U).transpose(1, 0, 2).reshape(128, DC * U))
    wpk[:, W2_OFF:W2_OFF + DC * U] = (
        W2_w.astype(npbf).reshape(DC, 128, U).transpose(1, 0, 2).reshape(128, DC * U))
    wpk[:, VB_OFF:VB_OFF + UC] = V_w.reshape(UC, 128).astype(npbf).T
    wpk[0:BL, IDN_OFF:IDN_OFF + BL] = np.eye(BL, dtype=npbf)
    wpk[0, BS_OFF:BS_OFF + U] = (W1_b + W2_b).astype(npbf)
    wpk[0, ONE_OFF:ONE_OFF + BL] = npbf(1.0)

    start = np.clip(pos.astype(np.int64) - HALF, 0, None)
    end = np.minimum(pos.astype(np.int64) + HALF, T)
    nvalid = np.clip(end - start, 0, L)
    lar = np.arange(L)
    tidx = np.clip(start[:, None] + lar[None, :], 0, T - 1)      # [B, L]
    mask = (lar[None, :] < nvalid[:, None]).astype(np.float32)   # [B, L]
    return wpk, tidx, mask


def kernel(**inputs):
    nc = _get_nc()

    query = np.asarray(inputs["query"], dtype=np.float32)
    values = np.asarray(inputs["values"], dtype=np.float32)
    pos = np.asarray(inputs["pos"])
    W1_w = np.asarray(inputs["W1_w"], dtype=np.float32)
    W1_b = np.asarray(inputs["W1_b"], dtype=np.float32)
    W2_w = np.asarray(inputs["W2_w"], dtype=np.float32)
    W2_b = np.asarray(inputs["W2_b"], dtype=np.float32)
    V_w = np.asarray(inputs["V_w"], dtype=np.float32)

    wpk_base, tidx, mask = _prep_shared(query, W1_w, W1_b, W2_w, W2_b, V_w, pos)

    vbf = values.astype(npbf)                                    # [T, B, D]
    bloc = np.arange(BL)

    in_maps = []
    for c in range(NCORES):
        bsl = slice(c * BL, (c + 1) * BL)
        vr = np.ascontiguousarray(vbf[:, bsl, :]).reshape(T * BL, D)
        rows = (tidx[bsl, :] * BL + bloc[:, None]).astype(np.int16)  # [16,128]
        flat = rows.reshape(-1)                                  # i = b*128 + l
        table = np.ascontiguousarray(flat.reshape(128, 16).T)    # [16, 128]
        table128 = np.tile(table, (8, 1))                        # replicated
        q = query[bsl].astype(npbf)                              # [16, 512]
        wpk = wpk_base.copy()
        wpk[:, QT_OFF:QT_OFF + DC * BL] = (
            q.reshape(BL, DC, 128).transpose(2, 1, 0).reshape(128, DC * BL))
        in_maps.append({
            "vrows": vr,
            "idxt": table128,
            "wpk": wpk,
            "mkp": np.ascontiguousarray(mask[bsl]),
        })

    res = run_bass_kernel_spmd(nc, in_maps, list(range(NCORES))).results
    ctxv = np.concatenate([r["out_ctx"] for r in res], axis=0)       # [B, D]
    attw = np.concatenate([r["out_w"] for r in res], axis=0)[..., None]
    return ctxv, attw


# revision 16
# speedup vs baseline: 1.2819x; 1.0339x over previous
"""Trainium2 Bass kernel for windowed (sparse) Bahdanau attention.

Reference computation (per batch b):
    window  = values[start_b : start_b+128, b, :]   (clipped to [0,T), zero-padded)
    qb1     = query[b] @ W2 + W1_b + W2_b
    score_l = V . tanh(window_l @ W1 + qb1)         (V_b cancels in softmax)
    w       = softmax(score)                        -> attention_weights output
    ctx     = sum_l w_l * window_l                  -> context_vector output

Strategy: data-parallel over batch across 8 NeuronCores (16 batches/core).
On each core the windows are fetched from HBM with SWDGE row gathers of the
bf16-cast values shard: transposed gathers land the windows with the D axis
on partitions (ready to be the moving operand of the W1 matmul, no on-chip
transposes), natural gathers provide [l, d] rows for the final context
matmul. Invalid (clipped / out-of-range) window rows are never masked in the
gather; instead their scores are patched to the exact "zero window" score
V.tanh(qb1) and their softmax weights are zeroed before the context matmul,
which reproduces the reference's zero-padding semantics exactly.

Score and context matmuls are M=1 rank-1 reductions; they are packed four to
a PSUM bank with tile_position col-tiling so their results can be drained
with a couple of wide [128, 512] copies and single DMAs instead of
single-partition ops.
"""

import os

import numpy as np
import ml_dtypes

from contextlib import ExitStack

import concourse.bass as bass
import concourse.bacc as bacc
import concourse.tile as tile
from concourse import mybir
from concourse.bass_utils import run_bass_kernel_spmd

B, T, D, U, L = 128, 2048, 512, 512, 128
NCORES = 8
BL = B // NCORES          # 16 local batches per core
DC = D // 128             # 4 contraction chunks
UC = U // 128             # 4 units chunks
NG = BL // 4              # 4 batch groups of 4 (N=512 matmul slices)
HALF = L // 2

# packed bf16 constants tensor layout (columns)
W1_OFF, W2_OFF = 0, 2048
QT_OFF, VB_OFF = 4096, 4160
IDN_OFF, BS_OFF, ONE_OFF = 4164, 4180, 4692
Z_OFF = 4708
WP_COLS = 4836

bf16 = mybir.dt.bfloat16
f32 = mybir.dt.float32
i16 = mybir.dt.int16
npbf = ml_dtypes.bfloat16
AF = mybir.ActivationFunctionType
ALU = mybir.AluOpType
AX = mybir.AxisListType


def _build_nc(reps=1):
    sim_safe = bool(os.environ.get("KERNEL_SIM_SAFE"))
    nc = bacc.Bacc("TRN2", target_bir_lowering=False, debug=False,
                   num_devices=NCORES)

    vrows = nc.dram_tensor("vrows", [T * BL, D], bf16, kind="ExternalInput").ap()
    idxt = nc.dram_tensor("idxt", [128, 128], i16, kind="ExternalInput").ap()
    wpk = nc.dram_tensor("wpk", [128, WP_COLS], bf16, kind="ExternalInput").ap()
    mkp = nc.dram_tensor("mkp", [BL, L], f32, kind="ExternalInput").ap()
    out_c = nc.dram_tensor("out_ctx", [BL, D], f32, kind="ExternalOutput").ap()
    out_w = nc.dram_tensor("out_w", [BL, L], f32, kind="ExternalOutput").ap()

    bench_loop = int(os.environ.get("BENCH_LOOP", "0"))

    with tile.TileContext(nc) as tc, ExitStack() as ctx:
        const = ctx.enter_context(tc.tile_pool(name="const", bufs=1))
        wp = const.tile([128, WP_COLS], bf16)
        nc.scalar.dma_start(wp[:], wpk)
        mk = const.tile([BL, L], f32)
        nc.scalar.dma_start(mk[:], mkp)
        idt = const.tile([128, 128], i16)
        nc.scalar.dma_start(idt[:], idxt)

        w1 = wp[:, W1_OFF:W1_OFF + DC * U]
        w2 = wp[:, W2_OFF:W2_OFF + DC * U]
        qt = wp[:, QT_OFF:QT_OFF + DC * BL]
        vb = wp[:, VB_OFF:VB_OFF + UC]
        idn = wp[0:BL, IDN_OFF:IDN_OFF + BL]
        ones = wp[0:1, ONE_OFF:ONE_OFF + BL]
        zrow = wp[0:1, Z_OFF:Z_OFF + 128]

        pools = dict(
            gat=ctx.enter_context(tc.tile_pool(name="gather", bufs=1)),
            qb1p=ctx.enter_context(tc.tile_pool(name="qb1", bufs=1)),
            psq=ctx.enter_context(tc.tile_pool(name="psq", bufs=1, space="PSUM")),
            xp=ctx.enter_context(tc.tile_pool(name="xpsum", bufs=4, space="PSUM")),
            sp=ctx.enter_context(tc.tile_pool(name="spsum", bufs=1, space="PSUM")),
            cp=ctx.enter_context(tc.tile_pool(name="cpsum", bufs=1, space="PSUM")),
            xs=ctx.enter_context(tc.tile_pool(name="xsb", bufs=4)),
            th=ctx.enter_context(tc.tile_pool(name="tanh", bufs=8)),
            sm=ctx.enter_context(tc.tile_pool(name="small", bufs=2)),
        )
        args = (nc, tc, pools, vrows, idt, w1, w2, qt, vb, idn, ones, zrow,
                wp, mk, out_c, out_w, sim_safe)
        if bench_loop:
            with tc.For_i(0, bench_loop, 1):
                _body(*args)
        else:
            for _ in range(reps):
                _body(*args)

    nc.compile()
    return nc


def _body(nc, tc, pools, vrows, idt, w1, w2, qt, vb, idn, ones, zrow,
          wp, mk, out_c, out_w, sim_safe):
    gat, qb1p, psq = pools["gat"], pools["qb1p"], pools["psq"]
    xp, sp, cp = pools["xp"], pools["sp"], pools["cp"]
    xs, th, sm = pools["xs"], pools["th"], pools["sm"]
    # windowed gathers: 2048 rows (16 batches x 128 window positions).
    # Split to keep each call within the single-packet descriptor limit
    # (transposed gathers >512 idxs / natural >1024 idxs crash the exec
    # unit with single_packet=True, and single_packet=False fires the
    # completion semaphore before the later packets finish).
    gTs = []   # per batch-group n: [d%128, (d//128, 4*128 (b,l))]
    for n in range(NG):
        gt = gat.tile([128, DC * 512], bf16, tag=f"gT{n}")
        nc.gpsimd.dma_gather(
            gt[:].rearrange("p (j n) -> p j n", j=DC), vrows,
            idt[:, n * 32:(n + 1) * 32], 512, 512, D, transpose=True)
        gTs.append(gt[:].rearrange("p (j n) -> p j n", j=DC))
    gNs = []   # per half h: [l, (b in 8, d)]
    for h in range(2):
        gn = gat.tile([128, 8 * D], bf16, tag=f"gN{h}")
        nc.gpsimd.dma_gather(
            gn[:].rearrange("p (b d) -> p b d", b=8), vrows,
            idt[:, h * 64:(h + 1) * 64], 1024, 1024, D)
        gNs.append(gn[:].rearrange("p (b d) -> p b d", b=8))

    # ---- q projection: qb1T[u, b] = sum_d W2[d,u] q[b,d] + (W1_b+W2_b)[u]
    qb1 = qb1p.tile([128, UC * BL], f32, tag="qb1")
    for i in range(UC):
        pq = psq.tile([128, BL], f32, tag="q")
        for j in range(DC):
            nc.tensor.matmul(pq[:],
                             w2[:, j * U + i * 128:j * U + i * 128 + 128],
                             qt[:, j * BL:(j + 1) * BL],
                             start=(j == 0), stop=False)
        nc.tensor.matmul(pq[:], wp[0:1, BS_OFF + i * 128:BS_OFF + (i + 1) * 128],
                         ones, start=False, stop=True)
        nc.any.tensor_copy(qb1[:, i * BL:(i + 1) * BL], pq[:])

    # ---- main matmul X.T[u, (b,l)] + qb1 broadcast + tanh; col-tiled V-dot
    ps4 = sp.tile([128, 512], f32, tag="s")   # scores: group n at partition 32n
    # zero-fill the bank so the col-tiled M=1 V-dots can accumulate into it
    # and the wide drain copy reads fully-initialized PSUM (only needed to
    # satisfy CoreSim's uninitialized-read check; hardware discards the
    # unwritten partitions at the strided output DMA)
    if sim_safe:
        nc.tensor.matmul(ps4[:], zrow, wp[0:1, 0:512], start=True, stop=False,
                         skip_group_check=True)
    for n in range(NG):
        tanhs = []
        for i in range(UC):
            px = xp.tile([128, 512], f32, tag="x")
            for j in range(DC):
                nc.tensor.matmul(px[:],
                                 w1[:, j * U + i * 128:j * U + i * 128 + 128],
                                 gTs[n][:, j, :],
                                 start=(j == 0), stop=(j == DC - 1))
            xb = xs.tile([128, 512], f32, tag="xb")
            qsl = qb1[:, i * BL + 4 * n:i * BL + 4 * n + 4]
            nc.vector.tensor_tensor(
                xb[:].rearrange("p (b l) -> p b l", b=4),
                px[:].rearrange("p (b l) -> p b l", b=4),
                qsl.unsqueeze(-1).broadcast_to((128, 4, 128)),
                op=ALU.add)
            tt = th.tile([128, 512], bf16, tag="t")
            nc.scalar.activation(tt[:], xb[:], AF.Tanh)
            tanhs.append(tt)
        for i in range(UC):
            nc.tensor.matmul(ps4[32 * n:32 * n + 1, :], vb[:, i:i + 1],
                             tanhs[i][:], start=(not sim_safe and i == 0),
                             stop=(n == NG - 1 and i == UC - 1),
                             tile_position=(0, 32 * n),
                             skip_group_check=True)

    # ---- invalid-row score: V . tanh(qb1)
    tq = sm.tile([128, UC * BL], bf16, tag="tq")
    nc.scalar.activation(tq[:], qb1[:], AF.Tanh)
    psv = sp.tile([1, BL], f32, tag="sv", bufs=1)
    for i in range(UC):
        nc.tensor.matmul(psv[:], vb[:, i:i + 1], tq[:, i * BL:(i + 1) * BL],
                         start=(i == 0), stop=(i == UC - 1))

    # ---- relayout scores to [b, l] (+ sinv in col 128) and run softmax
    s4 = sm.tile([128, 512], f32, tag="s4")
    nc.any.tensor_copy(s4[:], ps4[:])
    sveq = sm.tile([1, BL], f32, tag="sveq")
    nc.any.tensor_copy(sveq[:], psv[:])

    s_bt = sm.tile([BL, L + 1], f32, tag="sbt")
    nc.sync.dma_start(
        s_bt[:, 0:L],
        s4[:].rearrange("(q s) (b l) -> q s b l", s=32, b=4)[:, 0, :, :])
    nc.scalar.dma_start(s_bt[:, L:L + 1],
                        sveq[:].rearrange("o (b e) -> o b e", b=BL))
    sb = s_bt[:, 0:L]
    sv = s_bt[:, L:L + 1]

    spat = sm.tile([BL, L], f32, tag="spat")
    # (s - sinv) * mask + sinv  ==  mask ? s : sinv
    nc.vector.scalar_tensor_tensor(spat[:], sb, sv, mk[:],
                                   op0=ALU.subtract, op1=ALU.mult)
    nc.vector.tensor_scalar(spat[:], spat[:], sv, None, op0=ALU.add)

    mx = sm.tile([BL, 1], f32, tag="mx")
    nc.vector.tensor_reduce(mx[:], spat[:], axis=AX.X, op=ALU.max)
    nmx = sm.tile([BL, 1], f32, tag="nmx")
    nc.vector.tensor_scalar(nmx[:], mx[:], -1.0, None, op0=ALU.mult)
    ew = sm.tile([BL, L], f32, tag="ew")
    den = sm.tile([BL, 1], f32, tag="den")
    nc.scalar.activation(ew[:], spat[:], AF.Exp, bias=nmx[:], scale=1.0,
                         accum_out=den[:])
    rden = sm.tile([BL, 1], f32, tag="rden")
    nc.vector.reciprocal(rden[:], den[:])
    w_t = sm.tile([BL, L], f32, tag="wt")
    nc.vector.tensor_scalar(w_t[:], ew[:], rden[:], None, op0=ALU.mult)
    nc.sync.dma_start(out_w, w_t[:])

    # ---- masked weights, transposed, drive the col-tiled context matmuls
    wm = sm.tile([BL, L], bf16, tag="wm")
    nc.vector.tensor_tensor(wm[:], w_t[:], mk[:], op=ALU.mult)
    pwt = cp.tile([128, BL], bf16, tag="c", bufs=1)
    nc.tensor.transpose(pwt[:], wm[:], idn)
    wT = sm.tile([128, BL], bf16, tag="wT")
    nc.any.tensor_copy(wT[:], pwt[:])

    # batch b -> quad tile g = b % 4, partition 32*(b // 4)
    cstage = sm.tile([128, NG * 512], f32, tag="cst", bufs=1)
    for g in range(4):
        pc = cp.tile([128, 512], f32, tag="c", bufs=1)
        if sim_safe:
            nc.tensor.matmul(pc[:], zrow, wp[0:1, 0:512], start=True,
                             stop=False, skip_group_check=True)
        for c in range(4):
            b = 4 * c + g
            nc.tensor.matmul(pc[32 * c:32 * c + 1, :], wT[:, b:b + 1],
                             gNs[b // 8][:, b % 8, :],
                             start=(not sim_safe), stop=(c == 3),
                             tile_position=(0, 32 * c),
                             skip_group_check=True)
        nc.any.tensor_copy(cstage[:, g * 512:(g + 1) * 512], pc[:])
    nc.sync.dma_start(
        out_c,
        cstage[:].rearrange("(c s) (g d) -> c s g d", s=32, g=4)[:, 0, :, :])


_NC_CACHE = {}


def _get_nc():
    if "nc" not in _NC_CACHE:
        _NC_CACHE["nc"] = _build_nc(int(os.environ.get("KERNEL_REPS", "1")))
    return _NC_CACHE["nc"]


def _prep_shared(query, W1_w, W1_b, W2_w, W2_b, V_w, pos):
    """Small-tensor host prep shared across cores. Returns the packed
    constants tensor minus the per-core qt columns, plus gather metadata."""
    wpk = np.zeros((128, WP_COLS), dtype=npbf)
    wpk[:, W1_OFF:W1_OFF + DC * U] = (
        W1_w.astype(npbf).reshape(DC, 128, U).transpose(1, 0, 2).reshape(128, DC * U))
    wpk[:, W2_OFF:W2_OFF + DC * U] = (
        W2_w.astype(npbf).reshape(DC, 128, U).transpose(1, 0, 2).reshape(128, DC * U))
    wpk[:, VB_OFF:VB_OFF + UC] = V_w.reshape(UC, 128).astype(npbf).T
    wpk[0:BL, IDN_OFF:IDN_OFF + BL] = np.eye(BL, dtype=npbf)
    wpk[0, BS_OFF:BS_OFF + U] = (W1_b + W2_b).astype(npbf)
    wpk[0, ONE_OFF:ONE_OFF + BL] = npbf(1.0)

    start = np.clip(pos.astype(np.int64) - HALF, 0, None)
    end = np.minimum(pos.astype(np.int64) + HALF, T)
    nvalid = np.clip(end - start, 0, L)
    lar = np.arange(L)
    tidx = np.clip(start[:, None] + lar[None, :], 0, T - 1)      # [B, L]
    mask = (lar[None, :] < nvalid[:, None]).astype(np.float32)   # [B, L]
    return wpk, tidx, mask


def kernel(**inputs):
    nc = _get_nc()

    query = np.asarray(inputs["query"], dtype=np.float32)
    values = np.asarray(inputs["values"], dtype=np.float32)
    pos = np.asarray(inputs["pos"])
    W1_w = np.asarray(inputs["W1_w"], dtype=np.float32)
    W1_b = np.asarray(inputs["W1_b"], dtype=np.float32)
    W2_w = np.asarray(inputs["W2_w"], dtype=np.float32)
    W2_b = np.asarray(inputs["W2_b"], dtype=np.float32)
    V_w = np.asarray(inputs["V_w"], dtype=np.float32)

    wpk_base, tidx, mask = _prep_shared(query, W1_w, W1_b, W2_w, W2_b, V_w, pos)

    vbf = values.astype(npbf)                                    # [T, B, D]
    bloc = np.arange(BL)

    in_maps = []
    for c in range(NCORES):
        bsl = slice(c * BL, (c + 1) * BL)
        vr = np.ascontiguousarray(vbf[:, bsl, :]).reshape(T * BL, D)
        rows = (tidx[bsl, :] * BL + bloc[:, None]).astype(np.int16)  # [16,128]
        flat = rows.reshape(-1)                                  # i = b*128 + l
        table = np.ascontiguousarray(flat.reshape(128, 16).T)    # [16, 128]
        table128 = np.tile(table, (8, 1))                        # replicated
        q = query[bsl].astype(npbf)                              # [16, 512]
        wpk = wpk_base.copy()
        wpk[:, QT_OFF:QT_OFF + DC * BL] = (
            q.reshape(BL, DC, 128).transpose(2, 1, 0).reshape(128, DC * BL))
        in_maps.append({
            "vrows": vr,
            "idxt": table128,
            "wpk": wpk,
            "mkp": np.ascontiguousarray(mask[bsl]),
        })

    res = run_bass_kernel_spmd(nc, in_maps, list(range(NCORES))).results
    ctxv = np.concatenate([r["out_ctx"] for r in res], axis=0)       # [B, D]
    attw = np.concatenate([r["out_w"] for r in res], axis=0)[..., None]
    return ctxv, attw
